# revision 38
# baseline (speedup 1.0000x reference)
"""BatchedGraphSAGEDynamicRangeMean kernel for 8 Trainium2 NeuronCores.

Sharding: data-parallel over batch b -- core c computes graph c entirely
(N=4096 nodes, D=256), BN statistics are all-reduced across the 8 cores.

Per-core algorithm (one pass over 32 row-blocks of 128 nodes):
  - x loaded once via 8 batched DMAs into xsb (f32r); row L2 norms from
    Act square+accum; Xn^T built via PE transposes (f32r identity)
  - per block z: banded cosine sims S = XnT[:,center]^T @ XnT[:,cand]
    (f32r matmuls); left 128 cols = transpose of the previous block's
    masked sim rows (S symmetric, window validity symmetric)
  - window validity applied positionally with tensor_mask_reduce
    (per-partition [16*(r//16), 16*(r//16)+272) range, -FLT_MAX outside)
  - hardware max8 -> v3 = 3rd largest (top-1 is always self)
  - neighbor mask C = (S >= v3) - selfdiag in one scalar_tensor_tensor
  - x_neib*2 = C @ x_cand (f32r mask matmul); /2 folded into Wn
  - h1 = (Xn @ WxT)*norms*rinv, h2 = x_neibT^T @ (0.5*Wn)^T; row l2norm +
    relu fused into the PSUM->SBUF copies; h stored bf16
  - BN partial sums via ones-vector matmuls accumulated in PSUM
  - AllReduce(2x512) -> scale/bias rows -> broadcast via K=1 matmul ->
    apply (bf16 mult + f32 add), output staged 4 blocks per DMA
"""

import threading
import numpy as np

B, N, D, DOUT = 8, 4096, 256, 256
P = 128
NB = N // P            # 32 blocks
CAND = 3 * P           # 384 candidate columns per block
NCORES = 8
CH = 2 * DOUT          # 512 output channels
EPS_BN = 1e-5
EPS_NORM = 1e-12

_cache = {}
_lock = threading.Lock()


def _build(single=False, with_bias=False, sim_fp32=True):
    import concourse.bass as bass
    from concourse import bacc
    import concourse.mybir as mybir
    import concourse.tile as tile
    from concourse.masks import make_identity

    f32 = mybir.dt.float32
    f32r = mybir.dt.float32r
    bf16 = mybir.dt.bfloat16
    AF = mybir.ActivationFunctionType
    OP = mybir.AluOpType

    nc = bacc.Bacc("TRN2", target_bir_lowering=False)
    x_in = nc.declare_dram_parameter("xb", [N, D], f32, isOutput=False)
    wxT_in = nc.declare_dram_parameter("wxT", [D, DOUT], f32, isOutput=False)
    wnTh_in = nc.declare_dram_parameter("wnTh", [D, DOUT], f32, isOutput=False)
    gamma_in = nc.declare_dram_parameter("gamma", [1, CH], f32, isOutput=False)
    beta_in = nc.declare_dram_parameter("beta", [1, CH], f32, isOutput=False)
    wmint_in = nc.declare_dram_parameter("wmint", [P, CAND], f32, isOutput=False)
    wm0_in = nc.declare_dram_parameter("wm0", [P, CAND], f32, isOutput=False)
    wm31_in = nc.declare_dram_parameter("wm31", [P, CAND], f32, isOutput=False)
    if with_bias:
        bx_in = nc.declare_dram_parameter("bx", [1, DOUT], f32, isOutput=False)
        bn_in = nc.declare_dram_parameter("bn", [1, DOUT], f32, isOutput=False)
    out_ext = nc.declare_dram_parameter("out", [N, CH], f32, isOutput=True)

    simdt = f32 if sim_fp32 else f32r

    with tile.TileContext(nc) as tc:
        with (
            tc.tile_pool(name="persist", bufs=1) as pp,
            tc.tile_pool(name="work", bufs=2) as wp,
            tc.tile_pool(name="ps", bufs=1, space="PSUM") as ps,
            tc.tile_pool(name="ps2", bufs=2, space="PSUM") as ps2,
            tc.tile_pool(name="ps_st", bufs=1, space="PSUM") as ps_st,
            tc.tile_pool(name="xnrp", bufs=6) as xp,
            tc.tile_pool(name="dram", bufs=1, space="DRAM") as dp,
        ):
            # ---------------- persistent tensors ----------------
            xsb = pp.tile([P, NB + 2, D], f32)        # x rows, slot z+1 = block z
            xsb_bf = pp.tile([P, NB + 2, D], bf16)    # bf16 x for neighbor-sum lhsT
            xnT = pp.tile([P, 2, N + 2 * P], simdt)   # Xn^T, col = global_row+128
            hsb = pp.tile([P, NB, CH], bf16)          # h (pre-BN)
            identity_f = pp.tile([P, P], f32)         # base identity (f32)
            identity = pp.tile([P, P], simdt)         # transpose permutation
            identity_r = pp.tile([P, P], f32r)        # for f32r mask matmul
            identity_bf = pp.tile([P, P], bf16)       # for bf16 mask transposes
            diagS = pp.tile([P, CAND], f32)           # self one-hot at center
            wx = pp.tile([P, 2, DOUT], f32r)
            wn = pp.tile([P, 2, DOUT], f32r)
            gamma_row = pp.tile([1, CH], f32)
            beta_row = pp.tile([1, CH], f32)
            ones_row = pp.tile([1, P], f32r)
            ones_col = pp.tile([P, 1], bf16)
            wmint = pp.tile([P, CAND], f32r)          # window masks (NEG outside)
            wm0 = pp.tile([P, CAND], f32r)
            wm31 = pp.tile([P, CAND], f32r)
            norms = pp.tile([P, NB], f32)
            inv = pp.tile([P, NB], f32)
            ssq = pp.tile([P, NB], f32)
            sbc = pp.tile([P, CH], bf16)              # BN scale broadcast
            bbc = pp.tile([P, CH], bf16)              # BN bias broadcast
            rowb_r = pp.tile([1, CH], f32r)           # rounded rows for bc matmul
            rowd_r = pp.tile([1, CH], f32r)
            rowa = pp.tile([1, CH], f32)              # scratch rows
            rowb = pp.tile([1, CH], f32)
            rowc = pp.tile([1, CH], f32)
            rowd = pp.tile([1, CH], f32)
            eps_t = pp.tile([1, 1], f32)
            if with_bias:
                bx_row = pp.tile([1, DOUT], f32r)
                bn_row = pp.tile([1, DOUT], f32r)
                invT = pp.tile([NB, P], f32)
                invT2 = pp.tile([1, NB, P], f32r)

            make_identity(nc, identity_f)
            nc.vector.tensor_copy(identity, identity_f)
            nc.vector.tensor_copy(identity_r, identity_f)
            nc.vector.tensor_copy(identity_bf, identity_f)
            ones_f = pp.tile([P, P], f32)
            nc.gpsimd.memset(ones_f, 1.0)
            nc.vector.tensor_copy(ones_row, ones_f[0:1, :])
            nc.vector.tensor_copy(ones_col, ones_f[:, 0:1])
            nc.gpsimd.memset(eps_t, EPS_BN)
            zscr = pp.tile([P, D], f32)
            nc.gpsimd.memset(zscr, 0.0)
            nc.vector.tensor_copy(xsb[:, 0, :], zscr)
            nc.vector.tensor_copy(xsb[:, NB + 1, :], zscr)
            nc.vector.tensor_copy(xsb_bf[:, 0, :], zscr)
            nc.vector.tensor_copy(xsb_bf[:, NB + 1, :], zscr)
            for c in range(2):
                nc.vector.tensor_copy(xnT[:, c, 0:P], zscr[:, 0:P])
                nc.vector.tensor_copy(xnT[:, c, N + P:N + 2 * P], zscr[:, 0:P])
            nc.gpsimd.memset(diagS, 0.0)
            nc.vector.tensor_copy(diagS[:, P:2 * P], identity_f)

            nc.sync.dma_start(wmint, wmint_in[:, :].bitcast(f32r))
            nc.sync.dma_start(wm0, wm0_in[:, :].bitcast(f32r))
            nc.sync.dma_start(wm31, wm31_in[:, :].bitcast(f32r))
            for c in range(2):
                nc.sync.dma_start(wx[:, c, :],
                                  wxT_in[P * c:P * (c + 1), :].bitcast(f32r))
                nc.sync.dma_start(wn[:, c, :],
                                  wnTh_in[P * c:P * (c + 1), :].bitcast(f32r))
            nc.sync.dma_start(gamma_row, gamma_in[:, :])
            nc.sync.dma_start(beta_row, beta_in[:, :])
            if with_bias:
                nc.sync.dma_start(bx_row, bx_in[:, :].bitcast(f32r))
                nc.sync.dma_start(bn_row, bn_in[:, :].bitcast(f32r))

            # ---------------- batched x loads (4 blocks per DMA) ----------------
            for g in range(8):
                nc.sync.dma_start(
                    xsb[:, 1 + 4 * g:5 + 4 * g, :],
                    x_in[512 * g:512 * (g + 1), :]
                    .rearrange("(k p) d -> p k d", p=P))

            # ---------------- setup: norms, Xn^T (pipelined) ----------------
            xnr_blks = []
            def emit_setup(z):
                xv = xsb[:, z + 1, :]
                xsq = wp.tile([P, D], f32,
                              tag=("hcopy" if z % 2 == 0 else "hsq"))
                nc.scalar.activation(out=xsq, in_=xv,
                                     func=AF.Square, accum_out=ssq[:, z:z + 1])
                nc.scalar.activation(out=norms[:, z:z + 1], in_=ssq[:, z:z + 1],
                                     func=AF.Sqrt)
                nc.vector.reciprocal(out=inv[:, z:z + 1], in_=norms[:, z:z + 1])
                xn_blk = wp.tile([P, D], simdt,
                                 tag=("nb_sb" if z % 2 == 0 else "xt_sb"))
                nc.vector.tensor_scalar(out=xn_blk, in0=xv,
                                        scalar1=inv[:, z:z + 1], scalar2=None,
                                        op0=OP.mult)
                nc.gpsimd.tensor_copy(xsb_bf[:, z + 1, :], xv)
                tr_ps = ps.tile([P, 2, P], f32, tag="tr")
                for c in range(2):
                    nc.tensor.transpose(tr_ps[:, c, :].bitcast(simdt),
                                        xn_blk[:, P * c:P * (c + 1)], identity)
                ccol = P * (z + 1)
                nc.scalar.activation(out=xnT[:, :, ccol:ccol + P], in_=tr_ps,
                                     func=AF.Copy)
                xnr = xp.tile([P, 2, P], f32r, tag="xnr")
                nc.vector.tensor_copy(xnr, tr_ps)
                xnr_blks.append(xnr)
            if with_bias:
                trv_ps = ps.tile([NB, P], f32, tag="mt")
                nc.tensor.transpose(trv_ps, inv[:, 0:NB], identity)
                nc.vector.tensor_copy(invT, trv_ps)
                invT_d = dp.tile([NB, P], f32)
                nc.sync.dma_start(invT_d, invT)
                nc.sync.dma_start(
                    invT2,
                    invT_d[:, :].rearrange("a b -> (a b)")[None, :].bitcast(f32r))

            st_h = ps_st.tile([1, CH], f32, tag="sth")
            st_h2 = ps_st.tile([1, CH], f32, tag="sth2")
            def emit_main(z):
                cstart = P * (z + 1)

                # banded cosine sims; left 128 cols via transpose of the
                # previous block's masked right 128 cols (both S and the
                # window validity are symmetric)
                sim_ps = ps2.tile([P, CAND], f32, tag="sim")
                if z == 0:
                    for c in range(2):
                        nc.tensor.matmul(sim_ps, xnT[:, c, cstart:cstart + P],
                                         xnT[:, c, P * z:P * z + CAND],
                                         start=(c == 0), stop=False)
                    nc.tensor.matmul(sim_ps, identity_r, wm0,
                                     start=False, stop=True)
                else:
                    nc.tensor.transpose(sim_ps[:, 0:P].bitcast(simdt),
                                        prev_holder[0], identity)
                    wmask = wm31 if z == NB - 1 else wmint
                    for c in range(2):
                        nc.tensor.matmul(sim_ps[:, P:CAND],
                                         xnT[:, c, cstart:cstart + P],
                                         xnT[:, c, P * z + P:P * z + CAND],
                                         start=(c == 0), stop=False)
                    nc.tensor.matmul(sim_ps[:, P:CAND], identity_r,
                                     wmask[:, P:CAND], start=False, stop=True)
                if z < NB - 1:
                    prev_rs = wp.tile([P, P], simdt, tag="prevrs")
                    nc.vector.tensor_copy(prev_rs, sim_ps[:, 2 * P:CAND])
                    prev_holder[0] = prev_rs

                top8 = wp.tile([P, 8], f32, tag="top8")
                nc.vector.max(out=top8, in_=sim_ps)
                maskc = wp.tile([P, CAND], bf16, tag="maskc")
                nc.vector.scalar_tensor_tensor(out=maskc, in0=sim_ps,
                                               scalar=top8[:, 2:3],
                                               in1=diagS,
                                               op0=OP.is_ge, op1=OP.subtract)

                mt_ps = ps.tile([P, CAND], bf16, tag="mt")
                for k in range(3):
                    nc.tensor.transpose(mt_ps[:, P * k:P * (k + 1)],
                                        maskc[:, P * k:P * (k + 1)], identity_bf)
                mt_sb = wp.tile([P, CAND], bf16, tag="mt_sb")
                nc.vector.tensor_copy(mt_sb, mt_ps)

                # x_neib2^T = x_cand^T @ C^T directly: lhsT = natural x rows,
                # rhs = transposed bf16 mask chunks, accumulated over k
                xt_ps = ps.tile([P, 2, P], f32, tag="nb")
                for c in range(2):
                    for k in range(3):
                        nc.tensor.matmul(xt_ps[:, c, :],
                                         xsb_bf[:, z + k, P * c:P * (c + 1)],
                                         mt_sb[:, P * k:P * (k + 1)],
                                         start=(k == 0), stop=(k == 2))
                xt_sb = wp.tile([P, 2, P], f32r, tag="xt_sb")
                nc.vector.tensor_copy(xt_sb, xt_ps)

                # g1 = Xn @ WxT (+ inv*bx rank-1) ; h2 = x_neibT^T @ WnT_half
                gh_ps = ps.tile([P, 2, DOUT], f32, tag="g1")
                g1_ps = gh_ps[:, 0, :]
                h2_ps = gh_ps[:, 1, :]
                for c in range(2):
                    nc.tensor.matmul(g1_ps, xnr_blks[z][:, c, :],
                                     wx[:, c, :], start=(c == 0),
                                     stop=(c == 1 and not with_bias))
                if with_bias:
                    nc.tensor.matmul(g1_ps, invT2[:, z, :], bx_row,
                                     start=False, stop=True)
                for c in range(2):
                    nc.tensor.matmul(h2_ps, xt_sb[:, c, :], wn[:, c, :],
                                     start=(c == 0),
                                     stop=(c == 1 and not with_bias))
                if with_bias:
                    nc.tensor.matmul(h2_ps, ones_row, bn_row,
                                     start=False, stop=True)

                # fused l2norm + relu on the way out of PSUM
                sq_scr = wp.tile([P, DOUT], f32, tag="hsq")
                sA = wp.tile([P, 1], f32, tag="sA")
                nc.scalar.activation(out=sq_scr, in_=g1_ps, func=AF.Square,
                                     accum_out=sA)
                sq_scr2 = wp.tile([P, DOUT], f32, tag="hcopy")
                sB = wp.tile([P, 1], f32, tag="sB")
                nc.scalar.activation(out=sq_scr2, in_=h2_ps, func=AF.Square,
                                     accum_out=sB)
                # hno = sqrt(ssq*sA + sB), the concat-row l2 norm
                hno = wp.tile([P, 1], f32, tag="hno")
                nc.scalar.activation(out=hno, in_=sA, func=AF.Sqrt,
                                     scale=ssq[:, z:z + 1], bias=sB)
                rinv = wp.tile([P, 1], f32, tag="rinv")
                nc.vector.reciprocal(out=rinv, in_=hno)
                s1 = wp.tile([P, 1], f32, tag="s1")
                nc.gpsimd.tensor_mul(s1, norms[:, z:z + 1], rinv)
                nc.scalar.activation(out=hsb[:, z, 0:DOUT], in_=g1_ps,
                                     func=AF.Relu, scale=s1)
                nc.scalar.activation(out=hsb[:, z, DOUT:CH], in_=h2_ps,
                                     func=AF.Relu, scale=rinv)

                # BN partial sums (accumulated in PSUM across all blocks)
                hsq = wp.tile([P, CH], bf16, tag="hsqb")
                nc.gpsimd.tensor_mul(hsq, hsb[:, z, :], hsb[:, z, :])
                nc.tensor.matmul(st_h, ones_col, hsb[:, z, :],
                                 start=(z == 0), stop=(z == NB - 1))
                nc.tensor.matmul(st_h2, ones_col, hsq,
                                 start=(z == 0), stop=(z == NB - 1))

            prev_holder = [None]
            LA = 4
            for i in range(NB + LA):
                if i < NB:
                    emit_setup(i)
                if i >= LA:
                    emit_main(i - LA)

            # ---------------- BN stats all-reduce ----------------
            nc.vector.tensor_copy(rowa, st_h)
            nc.vector.tensor_copy(rowb, st_h2)
            st_in_d = dp.tile([2, CH], f32)
            st_out_d = dp.tile([2, CH], f32)
            nc.sync.dma_start(st_in_d[0:1, :], rowa)
            nc.sync.dma_start(st_in_d[1:2, :], rowb)
            if single:
                nc.sync.dma_start(st_out_d, st_in_d[:, :])
            else:
                nc.gpsimd.collective_compute(
                    "AllReduce", mybir.AluOpType.add,
                    replica_groups=[list(range(NCORES))],
                    ins=[st_in_d[:].opt()],
                    outs=[st_out_d[:].opt()],
                )
            nc.sync.dma_start(rowa, st_out_d[0:1, :])
            nc.sync.dma_start(rowb, st_out_d[1:2, :])
            sc = 1.0 / float(B * N)
            nc.vector.tensor_scalar_mul(rowa, rowa, sc)       # mu
            nc.vector.tensor_scalar_mul(rowb, rowb, sc)       # E[h^2]
            nc.vector.tensor_mul(rowc, rowa, rowa)            # mu^2
            nc.vector.tensor_sub(rowb, rowb, rowc)            # var
            nc.scalar.activation(out=rowb, in_=rowb, func=AF.Sqrt, bias=eps_t)
            nc.vector.reciprocal(out=rowb, in_=rowb)          # rstd
            nc.vector.tensor_mul(rowb, rowb, gamma_row)       # s = gamma*rstd
            nc.vector.tensor_mul(rowc, rowa, rowb)            # mu*s
            nc.vector.tensor_sub(rowd, beta_row, rowc)        # b = beta - mu*s

            # broadcast scale/bias rows to 128 partitions via K=1 matmul
            nc.vector.tensor_copy(rowb_r, rowb)
            nc.vector.tensor_copy(rowd_r, rowd)
            bc_ps = ps2.tile([P, CH], f32, tag="sim")
            nc.tensor.matmul(bc_ps, ones_row, rowb_r, start=True, stop=True)
            nc.vector.tensor_copy(sbc, bc_ps)
            bc_ps2 = ps.tile([P, CH], f32, tag="mt")
            nc.tensor.matmul(bc_ps2, ones_row, rowd_r, start=True, stop=True)
            nc.scalar.activation(out=bbc, in_=bc_ps2, func=AF.Copy)

            # ---------------- BN apply + writeback (2 blocks per DMA) ----------------
            for g in range(16):
                obuf = wp.tile([P, 2, CH], f32, tag="obuf")
                on_pool = (g % 3 == 2)
                for k in range(2):
                    z = 2 * g + k
                    tmp = wp.tile([P, CH], bf16, tag="hsqb")
                    if on_pool:
                        nc.gpsimd.tensor_mul(tmp, hsb[:, z, :], sbc)
                        nc.gpsimd.tensor_add(obuf[:, k, :], tmp, bbc)
                    else:
                        nc.vector.tensor_mul(tmp, hsb[:, z, :], sbc)
                        nc.vector.tensor_add(obuf[:, k, :], tmp, bbc)
                nc.sync.dma_start(
                    out_ext[256 * g:256 * (g + 1), :]
                    .rearrange("(k p) c -> p k c", p=P),
                    obuf)

    return _finish(nc)


def _finish(nc):
    nc.finalize()
    return nc


def _get_nc(**kw):
    key = tuple(sorted(kw.items()))
    with _lock:
        if key not in _cache:
            _cache[key] = _build(**kw)
        return _cache[key]


def _run(inputs, trace=False, trace_kwargs=None):
    from concourse.bass_utils import run_bass_kernel_spmd

    x = np.ascontiguousarray(np.asarray(inputs["x"], dtype=np.float32))
    Wx_w = np.asarray(inputs["Wx_w"], dtype=np.float32)
    Wx_b = np.asarray(inputs["Wx_b"], dtype=np.float32)
    Wn_w = np.asarray(inputs["Wn_w"], dtype=np.float32)
    Wn_b = np.asarray(inputs["Wn_b"], dtype=np.float32)
    gamma = np.asarray(inputs["gamma"], dtype=np.float32)
    beta = np.asarray(inputs["beta"], dtype=np.float32)
    assert x.shape == (B, N, D), x.shape
    assert int(inputs["p"]) == 16 and int(inputs["t"]) == 8

    with_bias = bool(np.any(Wx_b != 0.0) or np.any(Wn_b != 0.0))
    wxT = np.ascontiguousarray(Wx_w.T)
    wnTh = np.ascontiguousarray((0.5 * Wn_w).T)
    # window-validity masks in band coords (0 inside, NEG outside)
    NEG = -1.0e30
    r = np.arange(P)
    j16 = 16 * (r // 16)
    cols = np.arange(CAND)
    wmint = np.where((cols[None, :] >= j16[:, None])
                     & (cols[None, :] < j16[:, None] + 272), 0.0, NEG
                     ).astype(np.float32)
    wm0 = wmint.copy(); wm0[:, :P] = NEG
    wm31 = wmint.copy(); wm31[:, 2 * P:] = NEG
    shared = {
        "wxT": wxT, "wnTh": wnTh, "wmint": wmint, "wm0": wm0, "wm31": wm31,
        "gamma": gamma.reshape(1, CH), "beta": beta.reshape(1, CH),
    }
    if with_bias:
        shared["bx"] = Wx_b.reshape(1, DOUT)
        shared["bn"] = Wn_b.reshape(1, DOUT)
    in_maps = [{"xb": np.ascontiguousarray(x[c]), **shared} for c in range(NCORES)]

    nc = _get_nc(with_bias=with_bias)
    kw = {}
    if trace:
        kw = dict(trace=True, trace_kwargs=trace_kwargs or {})
    res = run_bass_kernel_spmd(nc, in_maps, core_ids=list(range(NCORES)), **kw)
    out = np.stack([res.results[c]["out"] for c in range(NCORES)], axis=0)
    return out.astype(np.float32), res


def kernel(**inputs):
    out, _ = _run(inputs)
    return out
